# revision 32
# baseline (speedup 1.0000x reference)
"""CompGCN layer kernel for Trainium2 (8 NeuronCores, SPMD).

Semantics (faithful to the reference, which replicates a torch
last-occurrence-wins index_matrix bug): for each half of the edge list,
every destination node d receives

    cnt[d] * (node_embed[src[last[d]]] + rela_embed[rela[last[d]]]) @ W.T

where last[d] is the LAST edge (highest index) with destination d and
cnt[d] the number of edges with destination d.  Plus a self-loop term
(node_embed + rela_embed[2*num_rela]) @ W_i.T, and h_r = rela_embed @ W_r.T.

Strategy: destination-sharded across the 8 cores.  The host does the O(E)
index bookkeeping (last/cnt via numpy scatter) and assembles the
per-destination message inputs A = cnt * (node[g] + rela[r]), shipped
TRANSPOSED ([feature, dest] layout, bf16) so the device streams contiguous
data straight into the tensor engine with zero on-chip transposes.  The
device performs the FLOP-heavy work: per 512-column chunk, three
accumulating 128x128 matmuls (bf16 inputs, fp32 PSUM accumulate) for the
W_i / W_o / W_s terms, the self-loop bias add fused into the PSUM->SBUF
copy (vector engine), plus the h_r matmul in fp32.  Output is streamed
back as h_v.T (bf16) and re-transposed to fp32 on the host.
"""

import sys

import numpy as np

for _p in ("/opt/trn_rl_repo",):
    if _p not in sys.path:
        sys.path.append(_p)

import ml_dtypes

import concourse.bass as bass
import concourse.mybir as mybir
import concourse.tile as tile
from concourse import bacc
from concourse.bass_utils import run_bass_kernel_spmd

N_NODES = 100000
NUM_RELA = 500
D = 128
N_CORES = 8
P = 128
SHARD = 12800           # padded per-core destination count (25 x 512)
ROWS = N_NODES // N_CORES  # 12500 real rows per core
CHUNK = 512             # psum bank width (fp32)
N_CHUNKS = SHARD // CHUNK  # 25
LOAD_CHUNKS = 8         # chunks per input DMA (3 MiB of packed bf16)
STORE_CHUNKS = 8        # chunks per output DMA (1 MiB in bf16)
RELA_PAD = 1008         # 1001 padded to 2x504 psum chunks

F32 = mybir.dt.float32
BF16 = mybir.dt.bfloat16
NP_BF16 = ml_dtypes.bfloat16

_NC_CACHE = {}


def _build_nc():
    nc = bacc.Bacc(None, target_bir_lowering=False)
    # xall: per 512-col chunk, [nodesT | aoT | asT] blocks interleaved
    xall = nc.dram_tensor("xall", [P, 3 * SHARD], BF16, kind="ExternalInput")
    wcat = nc.dram_tensor("wcat", [P, 3 * D], BF16, kind="ExternalInput")
    wrT = nc.dram_tensor("wrT", [P, D], F32, kind="ExternalInput")
    relaT = nc.dram_tensor("relaT", [P, RELA_PAD], F32, kind="ExternalInput")
    outT = nc.dram_tensor("outT", [P, SHARD], BF16, kind="ExternalOutput")
    hrT = nc.dram_tensor("hrT", [P, RELA_PAD], F32, kind="ExternalOutput")

    LW = LOAD_CHUNKS * CHUNK
    SW = STORE_CHUNKS * CHUNK

    with tile.TileContext(nc) as tc:
        with (
            tc.tile_pool(name="const", bufs=1) as cpool,
            tc.tile_pool(name="xin", bufs=3) as xpool,
            tc.tile_pool(name="outp", bufs=2) as opool,
            tc.tile_pool(name="hrp", bufs=1) as hpool,
            tc.tile_pool(name="ps", bufs=4, space="PSUM") as pspool,
            tc.tile_pool(name="psm", bufs=1, space="PSUM") as pmpool,
        ):
            wcat_sb = cpool.tile([P, 3 * D], BF16)
            nc.sync.dma_start(out=wcat_sb[:], in_=wcat[:])
            wrT_sb = cpool.tile([P, D], F32)
            nc.sync.dma_start(out=wrT_sb[:], in_=wrT[:])
            relaT_sb = cpool.tile([P, RELA_PAD], F32)
            nc.sync.dma_start(out=relaT_sb[:], in_=relaT[:])

            # load groups: small first so compute starts early
            groups = []
            _c = 0
            for g in (2, 3, 4):
                groups.append((_c, g))
                _c += g
            while _c < N_CHUNKS:
                g = min(LOAD_CHUNKS, N_CHUNKS - _c)
                groups.append((_c, g))
                _c += g
            chunk_group = {}
            for gi, (g0, gn) in enumerate(groups):
                for cc in range(g0, g0 + gn):
                    chunk_group[cc] = (gi, g0)

            # store groups: taper at the end so the last store is small
            sgroups = []
            _c = 0
            for g in (4, 8, 8, 3, 1, 1):
                sgroups.append((_c, g))
                _c += g
            assert _c == N_CHUNKS
            store_group = {}
            for g0, gn in sgroups:
                for cc in range(g0, g0 + gn):
                    store_group[cc] = (g0, gn)

            for c in range(N_CHUNKS):
                gi, g0 = chunk_group[c]
                if c == g0:
                    gn = groups[gi][1]
                    lw = 3 * gn * CHUNK
                    l0 = 3 * g0 * CHUNK
                    xt = xpool.tile([P, 3 * LW], BF16, tag="xt")
                    eng = nc.sync if gi % 2 == 0 else nc.scalar
                    eng.dma_start(out=xt[:, :lw], in_=xall[:, l0 : l0 + lw])
                sg0, sgn = store_group[c]
                if c == sg0:
                    ot = opool.tile([P, SW], BF16, tag="ot")
                b = (c - g0) * 3 * CHUNK

                po = pspool.tile([P, CHUNK], F32, tag="po")
                nc.tensor.matmul(
                    po[:], wcat_sb[:, 0:D], xt[:, b : b + CHUNK],
                    start=True, stop=False,
                )
                nc.tensor.matmul(
                    po[:], wcat_sb[:, D : 2 * D], xt[:, b + CHUNK : b + 2 * CHUNK],
                    start=False, stop=False,
                )
                nc.tensor.matmul(
                    po[:], wcat_sb[:, 2 * D : 3 * D], xt[:, b + 2 * CHUNK : b + 3 * CHUNK],
                    start=False, stop=True,
                )

                # evict psum -> bf16 out tile; alternate DVE / ACT (1:1)
                ob = (c - sg0) * CHUNK
                if c % 2 == 1:
                    nc.scalar.activation(
                        ot[:, ob : ob + CHUNK], po[:],
                        mybir.ActivationFunctionType.Copy,
                    )
                else:
                    nc.vector.tensor_copy(ot[:, ob : ob + CHUNK], po[:])
                if c == sg0 + sgn - 1:
                    sw = sgn * CHUNK
                    nc.scalar.dma_start(
                        out=outT[:, sg0 * CHUNK : sg0 * CHUNK + sw], in_=ot[:, :sw]
                    )

            # h_r = rela_embed @ W_r.T (fp32 path) — at the end, PE is warm
            for h0 in range(0, RELA_PAD, RELA_PAD // 2):
                hw = RELA_PAD // 2
                hr_ps = pmpool.tile([P, hw], F32, tag="hr")
                nc.tensor.matmul(
                    hr_ps[:], wrT_sb[:], relaT_sb[:, h0 : h0 + hw],
                    start=True, stop=True,
                )
                hr_sb = hpool.tile([P, hw], F32, tag="hr_sb")
                nc.vector.tensor_copy(hr_sb[:], hr_ps[:])
                nc.sync.dma_start(out=hrT[:, h0 : h0 + hw], in_=hr_sb[:])
    nc.compile()
    return nc


def _get_nc():
    if "nc" not in _NC_CACHE:
        _NC_CACHE["nc"] = _build_nc()
    return _NC_CACHE["nc"]


def _host_prep_half(node_embed, rela_embed, edges_half):
    """Per-destination message input: A = cnt * (node[g] + rela[r])."""
    e = edges_half.astype(np.int64, copy=False)
    src, rela, des = e[:, 0], e[:, 1], e[:, 2]
    n_e = e.shape[0]
    last = np.zeros(N_NODES, np.int64)
    last[des] = np.arange(n_e, dtype=np.int64)  # duplicate indices: last wins
    cnt = np.bincount(des, minlength=N_NODES).astype(np.float32)
    g = src[last]
    r = rela[last]
    A = cnt[:, None] * (node_embed[g] + rela_embed[r])
    return A


def _shard_inputs(node_embed, rela_embed, A_o, A_s, W_o, W_i, W_s, W_r):
    wcat = np.ascontiguousarray(
        np.concatenate([W_i.T, W_o.T, W_s.T], axis=1)
    ).astype(NP_BF16)
    wrT = np.ascontiguousarray(W_r.T)
    relaT = np.zeros((P, RELA_PAD), np.float32)
    relaT[:, : rela_embed.shape[0]] = rela_embed.T
    # self-loop composition folded into the node stream
    nodes_plus = node_embed + rela_embed[2 * NUM_RELA]

    in_maps = []
    for i in range(N_CORES):
        lo, hi = i * ROWS, (i + 1) * ROWS
        n = hi - lo
        # xall layout: [P, N_CHUNKS, 3, CHUNK] — per-chunk [nodesT | aoT | asT]
        xall = np.zeros((P, N_CHUNKS, 3, CHUNK), NP_BF16)
        for k, arr in ((0, nodes_plus), (1, A_o), (2, A_s)):
            tmp = np.zeros((P, SHARD), NP_BF16)
            tmp[:, :n] = arr[lo:hi].T.astype(NP_BF16)
            xall[:, :, k, :] = tmp.reshape(P, N_CHUNKS, CHUNK)
        xflat = xall.reshape(P, N_CHUNKS * 3 * CHUNK)
        in_maps.append(
            {
                "xall": xflat,
                "wcat": wcat,
                "wrT": wrT,
                "relaT": relaT,
            }
        )
    return in_maps


def kernel(**inputs):
    node_embed = np.asarray(inputs["node_embed"], dtype=np.float32)
    rela_embed = np.asarray(inputs["rela_embed"], dtype=np.float32)
    edges = np.asarray(inputs["edges"])
    W_o = np.asarray(inputs["W_o"], dtype=np.float32)
    W_i = np.asarray(inputs["W_i"], dtype=np.float32)
    W_s = np.asarray(inputs["W_s"], dtype=np.float32)
    W_r = np.asarray(inputs["W_r"], dtype=np.float32)

    half = edges.shape[0] // 2
    A_o = _host_prep_half(node_embed, rela_embed, edges[:half])
    A_s = _host_prep_half(node_embed, rela_embed, edges[half:])

    in_maps = _shard_inputs(node_embed, rela_embed, A_o, A_s, W_o, W_i, W_s, W_r)

    nc = _get_nc()
    res = run_bass_kernel_spmd(nc, in_maps, core_ids=list(range(N_CORES)))

    h_v = np.empty((N_NODES, D), np.float32)
    for i in range(N_CORES):
        outT = np.asarray(res.results[i]["outT"]).reshape(P, SHARD)
        h_v[i * ROWS : (i + 1) * ROWS] = outT[:, :ROWS].T.astype(np.float32)
    hrT = np.asarray(res.results[0]["hrT"]).reshape(P, RELA_PAD)
    h_r = np.ascontiguousarray(hrT[:, : rela_embed.shape[0]].T)
    return (h_v, h_r)


# revision 33
# speedup vs baseline: 1.2610x; 1.2610x over previous
"""CompGCN layer kernel for Trainium2 (8 NeuronCores, SPMD).

Semantics (faithful to the reference, which replicates a torch
last-occurrence-wins index_matrix bug): for each half of the edge list,
every destination node d receives

    cnt[d] * (node_embed[src[last[d]]] + rela_embed[rela[last[d]]]) @ W.T

where last[d] is the LAST edge (highest index) with destination d and
cnt[d] the number of edges with destination d.  Plus a self-loop term
(node_embed + rela_embed[2*num_rela]) @ W_i.T, and h_r = rela_embed @ W_r.T.

Strategy: destination-sharded across the 8 cores.  The host does the O(E)
index bookkeeping (last/cnt via numpy scatter) and assembles the
per-destination message inputs A = cnt * (node[g] + rela[r]), shipped
TRANSPOSED ([feature, dest] layout, bf16) so the device streams contiguous
data straight into the tensor engine with zero on-chip transposes.  The
device performs the FLOP-heavy work: per 512-column chunk, three
accumulating 128x128 matmuls (bf16 inputs, fp32 PSUM accumulate) for the
W_i / W_o / W_s terms, the self-loop bias add fused into the PSUM->SBUF
copy (vector engine), plus the h_r matmul in fp32.  Output is streamed
back as h_v.T (bf16) and re-transposed to fp32 on the host.
"""

import sys

import numpy as np

for _p in ("/opt/trn_rl_repo",):
    if _p not in sys.path:
        sys.path.append(_p)

import ml_dtypes

import concourse.bass as bass
import concourse.mybir as mybir
import concourse.tile as tile
from concourse import bacc
from concourse.bass_utils import run_bass_kernel_spmd

N_NODES = 100000
NUM_RELA = 500
D = 128
N_CORES = 8
P = 128
SHARD = 12800           # padded per-core destination count (25 x 512)
ROWS = N_NODES // N_CORES  # 12500 real rows per core
CHUNK = 512             # psum bank width (fp32)
N_CHUNKS = SHARD // CHUNK  # 25
LOAD_CHUNKS = 8         # chunks per input DMA (3 MiB of packed bf16)
STORE_CHUNKS = 8        # chunks per output DMA (1 MiB in bf16)
RELA_PAD = 1008         # 1001 padded to 2x504 psum chunks

F32 = mybir.dt.float32
BF16 = mybir.dt.bfloat16
NP_BF16 = ml_dtypes.bfloat16

_NC_CACHE = {}


def _build_nc():
    nc = bacc.Bacc(None, target_bir_lowering=False)
    # xall: per 512-col chunk, [nodesT | aoT | asT] blocks interleaved
    xall = nc.dram_tensor("xall", [P, 3 * SHARD], BF16, kind="ExternalInput")
    wcat = nc.dram_tensor("wcat", [P, 3 * D], BF16, kind="ExternalInput")
    wrT = nc.dram_tensor("wrT", [P, D], F32, kind="ExternalInput")
    relaT = nc.dram_tensor("relaT", [P, RELA_PAD], F32, kind="ExternalInput")
    outT = nc.dram_tensor("outT", [P, SHARD], BF16, kind="ExternalOutput")
    hrT = nc.dram_tensor("hrT", [P, RELA_PAD], F32, kind="ExternalOutput")

    LW = LOAD_CHUNKS * CHUNK
    SW = STORE_CHUNKS * CHUNK

    with tile.TileContext(nc) as tc:
        with (
            tc.tile_pool(name="const", bufs=1) as cpool,
            tc.tile_pool(name="xin", bufs=3) as xpool,
            tc.tile_pool(name="outp", bufs=2) as opool,
            tc.tile_pool(name="hrp", bufs=1) as hpool,
            tc.tile_pool(name="ps", bufs=4, space="PSUM") as pspool,
            tc.tile_pool(name="psm", bufs=1, space="PSUM") as pmpool,
        ):
            wcat_sb = cpool.tile([P, 3 * D], BF16)
            nc.sync.dma_start(out=wcat_sb[:], in_=wcat[:])
            wrT_sb = cpool.tile([P, D], F32)
            nc.sync.dma_start(out=wrT_sb[:], in_=wrT[:])
            relaT_sb = cpool.tile([P, RELA_PAD], F32)
            nc.sync.dma_start(out=relaT_sb[:], in_=relaT[:])

            # load groups: small first so compute starts early
            groups = []
            _c = 0
            for g in (2, 3, 4):
                groups.append((_c, g))
                _c += g
            while _c < N_CHUNKS:
                g = min(LOAD_CHUNKS, N_CHUNKS - _c)
                groups.append((_c, g))
                _c += g
            chunk_group = {}
            for gi, (g0, gn) in enumerate(groups):
                for cc in range(g0, g0 + gn):
                    chunk_group[cc] = (gi, g0)

            # store groups: taper at the end so the last store is small
            sgroups = []
            _c = 0
            for g in (4, 8, 8, 3, 1, 1):
                sgroups.append((_c, g))
                _c += g
            assert _c == N_CHUNKS
            store_group = {}
            for g0, gn in sgroups:
                for cc in range(g0, g0 + gn):
                    store_group[cc] = (g0, gn)

            for c in range(N_CHUNKS):
                gi, g0 = chunk_group[c]
                if c == g0:
                    gn = groups[gi][1]
                    lw = 3 * gn * CHUNK
                    l0 = 3 * g0 * CHUNK
                    xt = xpool.tile([P, 3 * LW], BF16, tag="xt")
                    nc.sync.dma_start(out=xt[:, :lw], in_=xall[:, l0 : l0 + lw])
                sg0, sgn = store_group[c]
                if c == sg0:
                    ot = opool.tile([P, SW], BF16, tag="ot")
                b = (c - g0) * 3 * CHUNK

                po = pspool.tile([P, CHUNK], F32, tag="po")
                nc.tensor.matmul(
                    po[:], wcat_sb[:, 0:D], xt[:, b : b + CHUNK],
                    start=True, stop=False,
                )
                nc.tensor.matmul(
                    po[:], wcat_sb[:, D : 2 * D], xt[:, b + CHUNK : b + 2 * CHUNK],
                    start=False, stop=False,
                )
                nc.tensor.matmul(
                    po[:], wcat_sb[:, 2 * D : 3 * D], xt[:, b + 2 * CHUNK : b + 3 * CHUNK],
                    start=False, stop=True,
                )

                # evict psum -> bf16 out tile; alternate DVE / ACT (1:1)
                ob = (c - sg0) * CHUNK
                if c % 2 == 1:
                    nc.scalar.activation(
                        ot[:, ob : ob + CHUNK], po[:],
                        mybir.ActivationFunctionType.Copy,
                    )
                else:
                    nc.vector.tensor_copy(ot[:, ob : ob + CHUNK], po[:])
                if c == sg0 + sgn - 1:
                    sw = sgn * CHUNK
                    nc.scalar.dma_start(
                        out=outT[:, sg0 * CHUNK : sg0 * CHUNK + sw], in_=ot[:, :sw]
                    )

            # h_r = rela_embed @ W_r.T (fp32 path) — at the end, PE is warm
            for h0 in range(0, RELA_PAD, RELA_PAD // 2):
                hw = RELA_PAD // 2
                hr_ps = pmpool.tile([P, hw], F32, tag="hr")
                nc.tensor.matmul(
                    hr_ps[:], wrT_sb[:], relaT_sb[:, h0 : h0 + hw],
                    start=True, stop=True,
                )
                hr_sb = hpool.tile([P, hw], F32, tag="hr_sb")
                nc.vector.tensor_copy(hr_sb[:], hr_ps[:])
                nc.sync.dma_start(out=hrT[:, h0 : h0 + hw], in_=hr_sb[:])
    nc.compile()
    return nc


def _get_nc():
    if "nc" not in _NC_CACHE:
        _NC_CACHE["nc"] = _build_nc()
    return _NC_CACHE["nc"]


def _host_prep_half(node_embed, rela_embed, edges_half):
    """Per-destination message input: A = cnt * (node[g] + rela[r])."""
    e = edges_half.astype(np.int64, copy=False)
    src, rela, des = e[:, 0], e[:, 1], e[:, 2]
    n_e = e.shape[0]
    last = np.zeros(N_NODES, np.int64)
    last[des] = np.arange(n_e, dtype=np.int64)  # duplicate indices: last wins
    cnt = np.bincount(des, minlength=N_NODES).astype(np.float32)
    g = src[last]
    r = rela[last]
    A = cnt[:, None] * (node_embed[g] + rela_embed[r])
    return A


def _shard_inputs(node_embed, rela_embed, A_o, A_s, W_o, W_i, W_s, W_r):
    wcat = np.ascontiguousarray(
        np.concatenate([W_i.T, W_o.T, W_s.T], axis=1)
    ).astype(NP_BF16)
    wrT = np.ascontiguousarray(W_r.T)
    relaT = np.zeros((P, RELA_PAD), np.float32)
    relaT[:, : rela_embed.shape[0]] = rela_embed.T
    # self-loop composition folded into the node stream
    nodes_plus = node_embed + rela_embed[2 * NUM_RELA]

    in_maps = []
    for i in range(N_CORES):
        lo, hi = i * ROWS, (i + 1) * ROWS
        n = hi - lo
        # xall layout: [P, N_CHUNKS, 3, CHUNK] — per-chunk [nodesT | aoT | asT]
        xall = np.zeros((P, N_CHUNKS, 3, CHUNK), NP_BF16)
        for k, arr in ((0, nodes_plus), (1, A_o), (2, A_s)):
            tmp = np.zeros((P, SHARD), NP_BF16)
            tmp[:, :n] = arr[lo:hi].T.astype(NP_BF16)
            xall[:, :, k, :] = tmp.reshape(P, N_CHUNKS, CHUNK)
        xflat = xall.reshape(P, N_CHUNKS * 3 * CHUNK)
        in_maps.append(
            {
                "xall": xflat,
                "wcat": wcat,
                "wrT": wrT,
                "relaT": relaT,
            }
        )
    return in_maps


def kernel(**inputs):
    node_embed = np.asarray(inputs["node_embed"], dtype=np.float32)
    rela_embed = np.asarray(inputs["rela_embed"], dtype=np.float32)
    edges = np.asarray(inputs["edges"])
    W_o = np.asarray(inputs["W_o"], dtype=np.float32)
    W_i = np.asarray(inputs["W_i"], dtype=np.float32)
    W_s = np.asarray(inputs["W_s"], dtype=np.float32)
    W_r = np.asarray(inputs["W_r"], dtype=np.float32)

    half = edges.shape[0] // 2
    A_o = _host_prep_half(node_embed, rela_embed, edges[:half])
    A_s = _host_prep_half(node_embed, rela_embed, edges[half:])

    in_maps = _shard_inputs(node_embed, rela_embed, A_o, A_s, W_o, W_i, W_s, W_r)

    nc = _get_nc()
    res = run_bass_kernel_spmd(nc, in_maps, core_ids=list(range(N_CORES)))

    h_v = np.empty((N_NODES, D), np.float32)
    for i in range(N_CORES):
        outT = np.asarray(res.results[i]["outT"]).reshape(P, SHARD)
        h_v[i * ROWS : (i + 1) * ROWS] = outT[:, :ROWS].T.astype(np.float32)
    hrT = np.asarray(res.results[0]["hrT"]).reshape(P, RELA_PAD)
    h_r = np.ascontiguousarray(hrT[:, : rela_embed.shape[0]].T)
    return (h_v, h_r)
